# revision 2
# baseline (speedup 1.0000x reference)
"""Trainium2 Bass kernel for the SPH composition loss (gnn_message_passing).

Spatial-hash variant of the row-sharded strategy: particles are Morton-
sorted by spatial cell; the 6144 query rows form 48 blocks of 128; each of
the 8 NeuronCores gets 6 blocks (size-balanced so all cores run an
identical instruction stream). For each block the pairwise SPH terms are
evaluated only against the particles within h of the block's query set
(exact ball-union, gathered+padded on host). Per-core partial loss sums
are combined on host ("all-reduce of the three scalar loss means").

Per-pair device math (fp32):
    d2   = relu(-2 x_j.x_i + sq_j + sq_i)        K=5 GEMM, clamp+scale drain
    t    = ln(d2/h^2 + EPS/h^2);  q = exp(.5 t);  iq = exp(-.5 t)
    u_   = min(q,1)-1;  v_ = min(q,.5)-.5        (negated clamped splines)
    u2   = u_^2;  v2q = (2 v_)^2
    W-sum   S1 = sum_j (u^3 - 4v^3)  via PE reduce of u2*u_ (-1s) + v2q*v_ (+1s)
    G    = u2 - v2q   (-dW/dq / 6)
    dot' = x_j.v_i + v_j.x_i - diag_j - diag_i   K=8 GEMM
    div     S2 = C * sum_j G*iq*dot', C = -6 sigma vol/h^2 (folded in PE weights)
"""
import sys
import os
sys.path.insert(0, "/opt/trn_rl_repo")
import numpy as np
from contextlib import ExitStack

NCORES = 8
BQ = 128            # queries per block
GRID = 9            # spatial grid per axis (cell side 1/9 >= h=0.11)
EPS = 1e-12

_PROGRAM_CACHE = {}
_last_results = None
# engine/structure options (model-tunable)
OPTS = {
    "dma": "sync",        # input DMA engine: gpsimd (SWDGE) | sync (HWDGE)
    "g_eng": "gpsimd",    # G = u2 - v2q
    "u3_eng": "gpsimd",   # u3n = u2*u_n
    "v3_eng": "gpsimd",   # v3n = v2q*v_n
    "t_eng": "vector",    # T = G*P
    "psmm_bufs": 2,
    "wp_bufs": 2,
    "sm_bufs": 2,
}


# ---------------------------------------------------------------- host prep
def _morton3(c):
    out = np.zeros(len(c), dtype=np.int64)
    for b in range(4):
        for d in range(3):
            out |= ((c[:, d] >> b) & 1) << (3 * b + d)
    return out


def _build_structure(pos, h):
    """Morton sort + per-block exact candidate lists (ball union)."""
    N = pos.shape[0]
    cell = np.clip(np.floor(pos * GRID).astype(np.int64), 0, GRID - 1)
    perm = np.argsort(_morton3(cell), kind="stable")
    pos_s = pos[perm]
    nblk = N // BQ
    cand_lists = []
    try:
        from scipy.spatial import cKDTree
        tree_all = cKDTree(pos_s)
        for b in range(nblk):
            qt = cKDTree(pos_s[b * BQ:(b + 1) * BQ])
            idx = qt.query_ball_tree(tree_all, r=float(h) * (1 + 1e-6))
            s = set()
            for lst in idx:
                s.update(lst)
            cand_lists.append(np.array(sorted(s), dtype=np.int64))
    except ImportError:
        # fallback: cell-union candidates
        cell_s = cell[perm]
        key_s = _morton3(cell_s)
        starts = {}
        for i in range(N):
            k = int(key_s[i])
            if k not in starts:
                starts[k] = [i, i + 1]
            else:
                starts[k][1] = i + 1
        for b in range(nblk):
            cells_here = np.unique(cell_s[b * BQ:(b + 1) * BQ], axis=0)
            ncells = set()
            for (cx, cy, cz) in cells_here:
                for dx in (-1, 0, 1):
                    for dy in (-1, 0, 1):
                        for dz in (-1, 0, 1):
                            nx, ny, nz = cx + dx, cy + dy, cz + dz
                            if 0 <= nx < GRID and 0 <= ny < GRID \
                                    and 0 <= nz < GRID:
                                ncells.add((nx, ny, nz))
            idx = []
            for cc in ncells:
                k = int(_morton3(np.array([cc]))[0])
                if k in starts:
                    idx.append(tuple(starts[k]))
            flat = np.concatenate([np.arange(s, e) for s, e in sorted(idx)])
            cand_lists.append(np.sort(flat))
    return perm, cand_lists


# ---------------------------------------------------------------- program
def _build_program(nseg_list, h, vol, n_rows_core, reps=1):
    import concourse.bass as bass
    import concourse.tile as tile
    from concourse import bacc, mybir, bass_isa
    from concourse.alu_op_type import AluOpType as alu

    f32 = mybir.dt.float32
    AF = mybir.ActivationFunctionType

    h = float(h)
    vol = float(vol)
    sigma = 8.0 / (np.pi * h ** 3)
    C = -6.0 * sigma * vol / h / h
    inv_h2 = 1.0 / (h * h)
    eps_q = EPS / (h * h)
    two_sig_vol = 2.0 * sigma * vol

    nblk = len(nseg_list)
    NQ = nblk * BQ
    assert NQ == n_rows_core
    l1w = (n_rows_core * 3) // 128
    nseg_max = max(nseg_list)

    # group blocks into wide-op groups (pair big with small; nseg_list is
    # sorted descending by construction)
    groups = [[b] for b in range(nblk)]

    nc = bacc.Bacc("TRN2", target_bir_lowering=False, debug=False,
                   num_devices=NCORES)
    # All activation funcs used here (Ln, Exp, Square, Abs, Relu, Copy,
    # Identity) live in the natural_log_exp_and_others table set. bass's
    # first-fit set picker would alternate natural_log <-> exp_and_others
    # per group (a ~2.7us table load each). Empty every other set in the
    # cached table dict (order preserved, so set ids stay valid) so all
    # activations resolve to the one superset -> a single table load.
    from concourse.hw_specs import get_activation_tables
    _tabs = get_activation_tables(nc.m.arch)
    if "natural_log_exp_and_others" in _tabs:
        for _k in list(_tabs.keys()):
            if _k != "natural_log_exp_and_others":
                _tabs[_k] = set()

    d_lhs_d2 = [nc.dram_tensor(f"lhs_d2_{b}", [5, nseg_list[b] * 128], f32,
                               kind="ExternalInput").ap() for b in range(nblk)]
    d_lhs_dot = [nc.dram_tensor(f"lhs_dot_{b}", [8, nseg_list[b] * 128], f32,
                                kind="ExternalInput").ap() for b in range(nblk)]
    d_rhs_d2 = nc.dram_tensor("rhs_d2", [5, NQ], f32, kind="ExternalInput").ap()
    d_rhs_dot = nc.dram_tensor("rhs_dot", [8, NQ], f32, kind="ExternalInput").ap()
    d_consts = nc.dram_tensor("consts", [128, 5], f32, kind="ExternalInput").ap()
    d_y = nc.dram_tensor("y_t", [128, l1w], f32, kind="ExternalInput").ap()
    d_pred = nc.dram_tensor("pred_t", [128, l1w], f32, kind="ExternalInput").ap()
    d_out = nc.dram_tensor("out", [1, 4], f32, kind="ExternalOutput").ap()

    es = ExitStack()
    with tile.TileContext(nc) as tc:
        with es:
            pin = es.enter_context(tc.tile_pool(name="pin", bufs=1))
            wp = es.enter_context(
                tc.tile_pool(name="wp", bufs=OPTS["wp_bufs"]))
            sm = es.enter_context(
                tc.tile_pool(name="sm", bufs=OPTS["sm_bufs"]))
            tail = es.enter_context(tc.tile_pool(name="tail", bufs=1))
            psmm = es.enter_context(
                tc.tile_pool(name="psmm", bufs=OPTS["psmm_bufs"],
                             space=bass.MemorySpace.PSUM))
            psacc = es.enter_context(
                tc.tile_pool(name="psacc", bufs=2, space=bass.MemorySpace.PSUM))

            # ---- input loads ----
            dmae = getattr(nc, OPTS["dma"])
            g_eng = getattr(nc, OPTS["g_eng"])
            u3_eng = getattr(nc, OPTS["u3_eng"])
            v3_eng = getattr(nc, OPTS["v3_eng"])
            t_eng = getattr(nc, OPTS["t_eng"])
            consts = pin.tile([128, 5], f32, tag="consts")
            dmae.dma_start(consts[:], d_consts)
            ONES = consts[:, 0:1]
            NEG1 = consts[:, 1:2]
            CVEC = consts[:, 2:3]
            EPSB = consts[:, 3:4]
            NEG1B = consts[0:1, 4:5]

            rhs_d2 = pin.tile([5, NQ], f32, tag="rhs_d2")
            dmae.dma_start(rhs_d2[:], d_rhs_d2)
            rhs_dot = pin.tile([8, NQ], f32, tag="rhs_dot")
            dmae.dma_start(rhs_dot[:], d_rhs_dot)

            lhs_d2_sb = []
            lhs_dot_sb = []
            for b in range(nblk):
                t = pin.tile([5, nseg_list[b] * 128], f32, tag=f"lhsd2_{b}")
                dmae.dma_start(t[:], d_lhs_d2[b])
                lhs_d2_sb.append(t)
                t = pin.tile([8, nseg_list[b] * 128], f32, tag=f"lhsdot_{b}")
                dmae.dma_start(t[:], d_lhs_dot[b])
                lhs_dot_sb.append(t)

            y_sb = pin.tile([128, l1w], f32, tag="y")
            dmae.dma_start(y_sb[:], d_y)
            pred_sb = pin.tile([128, l1w], f32, tag="pred")
            dmae.dma_start(pred_sb[:], d_pred)

            # ---- loss1 ----
            e_t = tail.tile([128, l1w], f32, tag="e")
            nc.vector.tensor_tensor(e_t[:], y_sb[:], pred_sb[:], alu.subtract)
            esq = tail.tile([128, l1w], f32, tag="esq")
            sq1 = tail.tile([128, 1], f32, tag="sq1")
            nc.scalar.activation(esq[:], e_t[:], AF.Square, accum_out=sq1[:])
            out_sb = tail.tile([1, 4], f32, tag="osb")
            nc.gpsimd.memset(out_sb[:], 0.0)
            l1pr = tail.tile([128, 1], f32, tag="l1pr")
            nc.gpsimd.partition_all_reduce(l1pr[:], sq1[:], 128,
                                           bass_isa.ReduceOp.add)
            nc.scalar.activation(out_sb[0:1, 0:1], l1pr[0:1, 0:1], AF.Copy)

            zb2 = tail.tile([1, NQ], f32, tag="zb2")
            zb3 = tail.tile([1, NQ], f32, tag="zb3")

            Fg_max = max(sum(nseg_list[b] for b in g) for g in groups) * BQ
            Fb_max = nseg_max * BQ

            # ---- main pass ----
            # reps via a hardware loop: NEFF size stays constant, so huge
            # rep counts (for reliable wall-clock marginal timing) compile
            # in the same time as reps=1.
            from contextlib import nullcontext
            loop_cm = tc.For_i(0, reps, 1) if reps > 1 else nullcontext()
            with loop_cm:
                # rep counter in out[0,3]: proves which NEFF actually ran
                nc.scalar.activation(out_sb[0:1, 3:4], out_sb[0:1, 3:4],
                                     AF.Identity, bias=1.0)
                for grp in groups:
                    Fg = sum(nseg_list[b] for b in grp) * BQ
                    boffs = {}
                    off = 0
                    for b in grp:
                        boffs[b] = off
                        off += nseg_list[b] * BQ

                    # d2 GEMMs + clamp/scale drains. SPB segments share a
                    # psum bank as one accumulation group; the explicit deps
                    # keep the start=True MM first in the PE FIFO.
                    from concourse.tile_rust import add_dep_helper
                    SPB = 512 // BQ
                    segs_g = [(b, s) for b in grp for s in range(nseg_list[b])]
                    d2c = wp.tile([128, Fg_max], f32, tag="A")
                    for g0 in range(0, len(segs_g), SPB):
                        chunk = segs_g[g0:g0 + SPB]
                        d2ps = psmm.tile([128, len(chunk) * BQ], f32,
                                         tag="d2ps")
                        prev = None
                        for ci, (b, s) in enumerate(chunk):
                            rq = slice(b * BQ, (b + 1) * BQ)
                            mm = nc.tensor.matmul(
                                d2ps[:, ci * BQ:(ci + 1) * BQ],
                                lhs_d2_sb[b][:, s * 128:(s + 1) * 128],
                                rhs_d2[:, rq], start=(ci == 0),
                                stop=(ci == len(chunk) - 1))
                            if prev is not None:
                                add_dep_helper(mm.ins, prev.ins, sync=False,
                                               reason="psum group MM order")
                            prev = mm
                        co = boffs[chunk[0][0]] + chunk[0][1] * BQ
                        nc.vector.tensor_scalar(
                            d2c[:, co:co + len(chunk) * BQ], d2ps[:],
                            0.0, inv_h2, alu.max, alu.mult)

                    t_ln = wp.tile([128, Fg_max], f32, tag="B")
                    nc.scalar.activation(t_ln[:, :Fg], d2c[:, :Fg], AF.Ln,
                                         bias=EPSB)
                    q_w = wp.tile([128, Fg_max], f32, tag="A")
                    nc.scalar.activation(q_w[:, :Fg], t_ln[:, :Fg], AF.Exp,
                                         scale=0.5)
                    iq = wp.tile([128, Fg_max], f32, tag="iq")
                    nc.scalar.activation(iq[:, :Fg], t_ln[:, :Fg], AF.Exp,
                                         scale=-0.5)
                    u_n = wp.tile([128, Fg_max], f32, tag="un")
                    nc.vector.tensor_scalar(u_n[:, :Fg], q_w[:, :Fg], 1.0, 1.0,
                                            alu.min, alu.subtract)
                    v_n = wp.tile([128, Fg_max], f32, tag="B")
                    nc.vector.tensor_scalar(v_n[:, :Fg], q_w[:, :Fg], 0.5, 0.5,
                                            alu.min, alu.subtract)
                    u2 = wp.tile([128, Fg_max], f32, tag="u2")
                    nc.scalar.activation(u2[:, :Fg], u_n[:, :Fg], AF.Square)
                    v2q = wp.tile([128, Fg_max], f32, tag="v2q")
                    nc.scalar.activation(v2q[:, :Fg], v_n[:, :Fg], AF.Square,
                                         scale=2.0)

                    # dot GEMMs + P = dot*iq drains (grouped psum banks)
                    P_w = wp.tile([128, Fg_max], f32, tag="P")
                    for g0 in range(0, len(segs_g), SPB):
                        chunk = segs_g[g0:g0 + SPB]
                        dotps = psmm.tile([128, len(chunk) * BQ], f32,
                                          tag="dotps")
                        prev = None
                        for ci, (b, s) in enumerate(chunk):
                            rq = slice(b * BQ, (b + 1) * BQ)
                            mm = nc.tensor.matmul(
                                dotps[:, ci * BQ:(ci + 1) * BQ],
                                lhs_dot_sb[b][:, s * 128:(s + 1) * 128],
                                rhs_dot[:, rq], start=(ci == 0),
                                stop=(ci == len(chunk) - 1))
                            if prev is not None:
                                add_dep_helper(mm.ins, prev.ins, sync=False,
                                               reason="psum group MM order")
                            prev = mm
                        co = boffs[chunk[0][0]] + chunk[0][1] * BQ
                        cw = len(chunk) * BQ
                        nc.vector.tensor_tensor(
                            P_w[:, co:co + cw], dotps[:],
                            iq[:, co:co + cw], alu.mult)

                    G_w = wp.tile([128, Fg_max], f32, tag="G")
                    g_eng.tensor_tensor(G_w[:, :Fg], u2[:, :Fg],
                                        v2q[:, :Fg], alu.subtract)
                    u3n_w = wp.tile([128, Fg_max], f32, tag="u3n")
                    u3_eng.tensor_tensor(u3n_w[:, :Fg], u2[:, :Fg],
                                         u_n[:, :Fg], alu.mult)
                    v3n_w = wp.tile([128, Fg_max], f32, tag="v3n")
                    v3_eng.tensor_tensor(v3n_w[:, :Fg], v2q[:, :Fg],
                                         v_n[:, :Fg], alu.mult)
                    T_w = wp.tile([128, Fg_max], f32, tag="T")
                    t_eng.tensor_tensor(T_w[:, :Fg], G_w[:, :Fg],
                                        P_w[:, :Fg], alu.mult)

                    # per-block PE reductions + |.| rows
                    for b in grp:
                        ns = nseg_list[b]
                        bo = boffs[b]
                        rq = slice(b * BQ, (b + 1) * BQ)
                        s1acc = psacc.tile([1, BQ], f32, tag="s1acc")
                        for s in range(ns):
                            cs = slice(bo + s * BQ, bo + (s + 1) * BQ)
                            nc.tensor.matmul(s1acc[:], NEG1, u3n_w[:, cs],
                                             start=(s == 0), stop=False)
                            nc.tensor.matmul(s1acc[:], ONES, v3n_w[:, cs],
                                             start=False, stop=(s == ns - 1))
                        s2acc = psacc.tile([1, BQ], f32, tag="s2acc")
                        for s in range(ns):
                            cs = slice(bo + s * BQ, bo + (s + 1) * BQ)
                            nc.tensor.matmul(s2acc[:], CVEC, T_w[:, cs],
                                             start=(s == 0),
                                             stop=(s == ns - 1))
                        nc.scalar.activation(zb2[0:1, rq], s1acc[:], AF.Abs,
                                             bias=NEG1B, scale=two_sig_vol)
                        nc.scalar.activation(zb3[0:1, rq], s2acc[:], AF.Abs)

            # ---- final partial sums ----
            nc.vector.tensor_reduce(out_sb[0:1, 1:2], zb2[:],
                                    mybir.AxisListType.X, alu.add)
            nc.vector.tensor_reduce(out_sb[0:1, 2:3], zb3[:],
                                    mybir.AxisListType.X, alu.add)
            nc.sync.dma_start(d_out, out_sb[:])
    nc.compile()
    return nc


# ---------------------------------------------------------------- kernel
def prepare(inputs, reps=1):
    """Build (nc, in_maps, N) for the given inputs."""
    pred = np.asarray(inputs["pred"], dtype=np.float32)
    y = np.asarray(inputs["y"], dtype=np.float32)
    mid_pos = np.asarray(inputs["mid_pos"], dtype=np.float32)
    mid_vel = np.asarray(inputs["mid_vel"], dtype=np.float32)
    y_mean = np.asarray(inputs["y_mean"], dtype=np.float32)
    y_std = np.asarray(inputs["y_std"], dtype=np.float32)
    h = np.float32(inputs["h"])
    vol = np.float32(inputs["vol"])
    dt = np.float32(inputs["dt"])
    nb = int(inputs["num_boundary_particles"])
    N = pred.shape[0]
    rows_core = N // NCORES

    y_inv = (y * y_std + y_mean).astype(np.float32)
    pos = mid_pos.copy()
    pos[nb:] += y_inv[nb:]
    vel = mid_vel.copy()
    vel[nb:] += (y_inv[nb:] / dt).astype(np.float32)
    sq = np.sum(pos * pos, axis=1, dtype=np.float32)
    diag = np.sum(pos * vel, axis=1, dtype=np.float32)

    perm, cand_lists = _build_structure(pos, h)
    pos_s = pos[perm]; vel_s = vel[perm]
    sq_s = sq[perm]; diag_s = diag[perm]
    y_s = y[perm]; pred_s = pred[perm]

    nblk_total = N // BQ
    nblk_core = nblk_total // NCORES
    # size-balanced slot assignment: slot k gets the k-th octile by size
    order = np.argsort([-len(c) for c in cand_lists], kind="stable")
    slots = [order[k * NCORES:(k + 1) * NCORES] for k in range(nblk_core)]
    nseg_list = []
    for k in range(nblk_core):
        mx = max(len(cand_lists[b]) for b in slots[k])
        nseg_list.append(int(np.ceil(mx / 128)))

    key = (tuple(nseg_list), float(h), float(vol), N, reps)
    if key not in _PROGRAM_CACHE:
        _PROGRAM_CACHE[key] = _build_program(nseg_list, h, vol, rows_core,
                                             reps=reps)
    nc = _PROGRAM_CACHE[key]

    sigma = 8.0 / (np.pi * float(h) ** 3)
    C = -6.0 * sigma * float(vol) / float(h) ** 2
    l1w = (rows_core * 3) // 128

    in_maps = []
    for c in range(NCORES):
        m = {}
        qsel = []
        for k in range(nblk_core):
            b = int(slots[k][c])
            qsel.append(np.arange(b * BQ, (b + 1) * BQ))
            ci = cand_lists[b]
            L = nseg_list[k] * 128
            npad = L - len(ci)
            cpos = np.concatenate([pos_s[ci],
                                   np.full((npad, 3), 1e3, np.float32)])
            cvel = np.concatenate([vel_s[ci],
                                   np.zeros((npad, 3), np.float32)])
            csq = np.concatenate([sq_s[ci], np.full(npad, 3e6, np.float32)])
            cdiag = np.concatenate([diag_s[ci], np.zeros(npad, np.float32)])
            lhs_d2 = np.empty((5, L), np.float32)
            lhs_d2[0:3] = -2.0 * cpos.T
            lhs_d2[3] = csq
            lhs_d2[4] = 1.0
            lhs_dot = np.empty((8, L), np.float32)
            lhs_dot[0:3] = cpos.T
            lhs_dot[3:6] = cvel.T
            lhs_dot[6] = cdiag
            lhs_dot[7] = 1.0
            m[f"lhs_d2_{k}"] = lhs_d2
            m[f"lhs_dot_{k}"] = lhs_dot
        qidx = np.concatenate(qsel)
        qpos = pos_s[qidx]; qvel = vel_s[qidx]
        qsq = sq_s[qidx]; qdiag = diag_s[qidx]
        NQ = rows_core
        rhs_d2 = np.empty((5, NQ), np.float32)
        rhs_d2[0:3] = qpos.T
        rhs_d2[3] = 1.0
        rhs_d2[4] = qsq
        rhs_dot = np.empty((8, NQ), np.float32)
        rhs_dot[0:3] = qvel.T
        rhs_dot[3:6] = qpos.T
        rhs_dot[6] = -1.0
        rhs_dot[7] = -qdiag
        m["rhs_d2"] = rhs_d2
        m["rhs_dot"] = rhs_dot
        consts = np.empty((128, 5), np.float32)
        consts[:, 0] = 1.0
        consts[:, 1] = -1.0
        consts[:, 2] = C
        consts[:, 3] = EPS / (float(h) * float(h))
        consts[:, 4] = -1.0
        m["consts"] = consts
        rr = slice(c * rows_core, (c + 1) * rows_core)
        m["y_t"] = y_s[rr].reshape(128, l1w)
        m["pred_t"] = pred_s[rr].reshape(128, l1w)
        in_maps.append(m)
    return nc, in_maps, N


def combine(results, N):
    parts = np.stack([results[c]["out"][0] for c in range(NCORES)])
    l1 = float(np.sum(parts[:, 0], dtype=np.float64))
    l2 = float(np.sum(parts[:, 1], dtype=np.float64))
    l3 = float(np.sum(parts[:, 2], dtype=np.float64))
    total = np.float32(1.0 * l1 / N) + np.float32(0.1) * np.float32(l2 / N) \
        + np.float32(0.1) * np.float32(l3 / N)
    return np.array(total, dtype=np.float32)


def kernel(**inputs):
    from concourse.bass_utils import run_bass_kernel_spmd
    nc, in_maps, N = prepare(inputs)
    res = run_bass_kernel_spmd(nc, in_maps, core_ids=list(range(NCORES)))
    global _last_results
    _last_results = res
    return combine(res.results, N)



# revision 8
# speedup vs baseline: 1.1195x; 1.1195x over previous
"""Trainium2 Bass kernel for the SPH composition loss (gnn_message_passing).

Spatial-hash row-sharded strategy: particles are Morton-sorted by spatial
cell; the 6144 query rows form 48 blocks of 128; each of the 8 NeuronCores
gets 6 blocks (size-balanced so all cores run an identical instruction
stream). For each block the pairwise SPH terms are evaluated only against
the particles within h of the block's query set (exact ball-union,
gathered+padded on host). Per-core partial loss sums are combined on host
("all-reduce of the three scalar loss means").

v2 design (fp16 GEMMs + single-activation chain):
  - positions/velocities are BLOCK-LOCAL (centered on the candidate-set
    mean) and h-scaled, so fp16 GEMMs carry enough precision and
    d2ps = q^2 + b comes straight out of the PE (scale+bias folded into
    the GEMM weight rows).
  - q    = Sqrt(d2ps)                       [Act, direct PSUM read]
  - P    = dotps / q                        [Pool divide, direct PSUM read]
  - u_   = min(q,1)-1;  v_ = min(q,.5)-.5   [DVE 4x fp16]
  - u2   = u_^2 [Act Square]; v2q = 4 v_^2  [DVE]
  - G    = u2 - v2q;  T = G*P               [DVE]
  - u3n  = u2*u_;  v3n = v2q*v_             [Pool]
  - WX   = v3n - u3n  (= 4v^3 - u^3)        [DVE]
  - PE reduce streams (fp16 1cyc/row): s1 = sum tsv*WX, s2 = sum Ch*T
  - rho/rho0 = s1;  zb2 = s1 - 1;  zb3 = s2; |.| folds into the final
    tensor_reduce(apply_absolute_value=True).
Reps for timing run inside a tc.For_i hardware loop (NEFF size constant).
"""
import sys
import os
sys.path.insert(0, "/opt/trn_rl_repo")
import numpy as np
from contextlib import ExitStack, nullcontext

NCORES = 8
BQ = 128            # queries per block
GRID = 9            # spatial grid per axis (cell side 1/9 >= h=0.11)
QB = 1e-4           # bias on q^2 (hi/lo-split GEMM noise is ~2e-5)
PAD_X = 100.0       # padding coord in local/h units (q ~ 170 >> 1)

_PROGRAM_CACHE = {}
_last_results = None
OPTS = {
    "dma": "sync",
    "u2_eng": "scalar",    # u2 = Square(u_)
    "v2_eng": "vector",    # v2 = v_*v_
    "u3_eng": "gpsimd",    # u3n = u2*u_
    "v3_eng": "gpsimd",    # v3n = v2q*v_
    "p_eng": "vector",     # P = dotps * iq  (PSUM read -> DVE only)
    "wx_eng": "gpsimd",    # WX = v3n - u3n
    "t_eng": "vector",     # T = G*P
    "g_eng": "gpsimd",     # G = u2 - v2q
    "psmm_bufs": 2,
    "wp_bufs": 2,
}


# ---------------------------------------------------------------- host prep
def _morton3(c):
    out = np.zeros(len(c), dtype=np.int64)
    for b in range(4):
        for d in range(3):
            out |= ((c[:, d] >> b) & 1) << (3 * b + d)
    return out


def _build_structure(pos, h):
    """Morton sort + per-block exact candidate lists (ball union)."""
    N = pos.shape[0]
    cell = np.clip(np.floor(pos * GRID).astype(np.int64), 0, GRID - 1)
    perm = np.argsort(_morton3(cell), kind="stable")
    pos_s = pos[perm]
    nblk = N // BQ
    cand_lists = []
    try:
        from scipy.spatial import cKDTree
        tree_all = cKDTree(pos_s)
        for b in range(nblk):
            qt = cKDTree(pos_s[b * BQ:(b + 1) * BQ])
            idx = qt.query_ball_tree(tree_all, r=float(h) * (1 + 1e-6))
            s = set()
            for lst in idx:
                s.update(lst)
            cand_lists.append(np.array(sorted(s), dtype=np.int64))
    except ImportError:
        # exact ball-union via brute-force blockwise distances
        rr = (float(h) * (1 + 1e-6)) ** 2
        for b in range(nblk):
            qp = pos_s[b * BQ:(b + 1) * BQ]
            d2 = ((qp[:, None, :] - pos_s[None, :, :]) ** 2).sum(-1)
            cand_lists.append(np.nonzero((d2 <= rr).any(axis=0))[0]
                              .astype(np.int64))
    return perm, cand_lists


# ---------------------------------------------------------------- program
def _build_program(nseg_list, h, vol, n_rows_core, reps=1):
    import concourse.bass as bass
    import concourse.tile as tile
    from concourse import bacc, mybir, bass_isa
    from concourse.alu_op_type import AluOpType as alu
    from concourse.tile_rust import add_dep_helper

    f32 = mybir.dt.float32
    f16 = mybir.dt.float16
    AF = mybir.ActivationFunctionType

    h = float(h)
    vol = float(vol)
    sigma = 8.0 / (np.pi * h ** 3)
    tsv = 2.0 * sigma * vol            # two_sig_vol: rho/rho0 = tsv*S1
    ch = -6.0 * sigma * vol / h        # s2 weight: div = sum ch*G*(dot/h)/q

    nblk = len(nseg_list)
    NQ = nblk * BQ
    assert NQ == n_rows_core
    l1w = (n_rows_core * 3) // 128
    nseg_max = max(nseg_list)
    SPB = 4                            # segs per psum chunk (4*128 = 512)

    nc = bacc.Bacc("TRN2", target_bir_lowering=False, debug=False,
                   num_devices=NCORES)
    # All activations used (Sqrt, Square, Copy, Identity) live in
    # sqrt_and_friends; empty the other cached table sets (order preserved)
    # so the first-fit picker resolves everything to one table -> a single
    # ~1.3us table load.
    from concourse.hw_specs import get_activation_tables
    _tabs = get_activation_tables(nc.m.arch)
    if "sqrt_and_friends" in _tabs:
        for _k in list(_tabs.keys()):
            if _k != "sqrt_and_friends":
                _tabs[_k] = set()

    d_lhs_d2 = [nc.dram_tensor(f"lhs_d2_{b}", [13, nseg_list[b] * 128], f16,
                               kind="ExternalInput").ap() for b in range(nblk)]
    d_lhs_dot = [nc.dram_tensor(f"lhs_dot_{b}", [8, nseg_list[b] * 128], f16,
                                kind="ExternalInput").ap() for b in range(nblk)]
    d_rhs_d2 = nc.dram_tensor("rhs_d2", [13, NQ], f16,
                              kind="ExternalInput").ap()
    d_rhs_dot = nc.dram_tensor("rhs_dot", [8, NQ], f16,
                               kind="ExternalInput").ap()
    d_wcol = nc.dram_tensor("wcol", [128, 3], f16, kind="ExternalInput").ap()
    d_y = nc.dram_tensor("y_t", [128, l1w], f32, kind="ExternalInput").ap()
    d_pred = nc.dram_tensor("pred_t", [128, l1w], f32,
                            kind="ExternalInput").ap()
    d_out = nc.dram_tensor("out", [1, 4], f32, kind="ExternalOutput").ap()

    es = ExitStack()
    with tile.TileContext(nc) as tc:
        with es:
            pin = es.enter_context(tc.tile_pool(name="pin", bufs=1))
            wp = es.enter_context(
                tc.tile_pool(name="wp", bufs=OPTS["wp_bufs"]))
            tail = es.enter_context(tc.tile_pool(name="tail", bufs=1))
            psmm = es.enter_context(
                tc.tile_pool(name="psmm", bufs=OPTS["psmm_bufs"],
                             space=bass.MemorySpace.PSUM))
            psacc = es.enter_context(
                tc.tile_pool(name="psacc", bufs=2, space=bass.MemorySpace.PSUM))

            dmae = getattr(nc, OPTS["dma"])
            u2_eng = getattr(nc, OPTS["u2_eng"])
            v2_eng = getattr(nc, OPTS["v2_eng"])
            u3_eng = getattr(nc, OPTS["u3_eng"])
            v3_eng = getattr(nc, OPTS["v3_eng"])
            p_eng = getattr(nc, OPTS["p_eng"])
            wx_eng = getattr(nc, OPTS["wx_eng"])
            t_eng = getattr(nc, OPTS["t_eng"])
            g_eng = getattr(nc, OPTS["g_eng"])

            # ---- input loads ----
            wcol = pin.tile([128, 3], f16, tag="wcol")
            dmae.dma_start(wcol[:], d_wcol)
            W_U3 = wcol[:, 0:1]     # -tsv
            W_V3 = wcol[:, 1:2]     # +tsv   (s1 via WX uses W_V3 only)
            W_CH = wcol[:, 2:3]     # ch

            rhs_d2 = pin.tile([13, NQ], f16, tag="rhs_d2")
            dmae.dma_start(rhs_d2[:], d_rhs_d2)
            rhs_dot = pin.tile([8, NQ], f16, tag="rhs_dot")
            dmae.dma_start(rhs_dot[:], d_rhs_dot)

            lhs_d2_sb = []
            lhs_dot_sb = []
            for b in range(nblk):
                t = pin.tile([13, nseg_list[b] * 128], f16, tag=f"lhsd2_{b}")
                dmae.dma_start(t[:], d_lhs_d2[b])
                lhs_d2_sb.append(t)
                t = pin.tile([8, nseg_list[b] * 128], f16, tag=f"lhsdot_{b}")
                dmae.dma_start(t[:], d_lhs_dot[b])
                lhs_dot_sb.append(t)

            y_sb = pin.tile([128, l1w], f32, tag="y")
            dmae.dma_start(y_sb[:], d_y)
            pred_sb = pin.tile([128, l1w], f32, tag="pred")
            dmae.dma_start(pred_sb[:], d_pred)

            # ---- loss1 ----
            e_t = tail.tile([128, l1w], f32, tag="e")
            nc.vector.tensor_tensor(e_t[:], y_sb[:], pred_sb[:], alu.subtract)
            esq = tail.tile([128, l1w], f32, tag="esq")
            sq1 = tail.tile([128, 1], f32, tag="sq1")
            nc.scalar.activation(esq[:], e_t[:], AF.Square, accum_out=sq1[:])
            out_sb = tail.tile([1, 4], f32, tag="osb")
            nc.gpsimd.memset(out_sb[:], 0.0)
            l1pr = tail.tile([128, 1], f32, tag="l1pr")
            nc.gpsimd.partition_all_reduce(l1pr[:], sq1[:], 128,
                                           bass_isa.ReduceOp.add)
            nc.scalar.activation(out_sb[0:1, 0:1], l1pr[0:1, 0:1], AF.Copy)

            zb2 = tail.tile([1, NQ], f32, tag="zb2")
            zb3 = tail.tile([1, NQ], f32, tag="zb3")

            Fb_max = nseg_max * BQ

            # ---- main pass (hw loop for timing reps) ----
            loop_cm = tc.For_i(0, reps, 1) if reps > 1 else nullcontext()
            with loop_cm:
                # rep counter in out[0,3]: proves which NEFF actually ran
                nc.scalar.activation(out_sb[0:1, 3:4], out_sb[0:1, 3:4],
                                     AF.Identity, bias=1.0)
                for b in range(nblk):
                    ns = nseg_list[b]
                    Fb = ns * BQ
                    rq = slice(b * BQ, (b + 1) * BQ)

                    q_w = wp.tile([128, Fb_max], f16, tag="q")
                    iq_w = wp.tile([128, Fb_max], f32, tag="iq")
                    P_w = wp.tile([128, Fb_max], f16, tag="P")
                    # psum chunks of SPB segs: GEMMs + q/P drains
                    for c0 in range(0, ns, SPB):
                        csegs = list(range(c0, min(c0 + SPB, ns)))
                        cw = len(csegs) * BQ
                        co = c0 * BQ
                        d2ps = psmm.tile([128, cw], f32, tag="d2ps")
                        prev = None
                        for ci, s in enumerate(csegs):
                            mm = nc.tensor.matmul(
                                d2ps[:, ci * BQ:(ci + 1) * BQ],
                                lhs_d2_sb[b][:, s * 128:(s + 1) * 128],
                                rhs_d2[:, rq], start=(ci == 0),
                                stop=(ci == len(csegs) - 1))
                            if prev is not None:
                                add_dep_helper(mm.ins, prev.ins, sync=False,
                                               reason="psum group MM order")
                            prev = mm
                        nc.scalar.activation(q_w[:, co:co + cw], d2ps[:],
                                             AF.Sqrt)
                        nc.vector.reciprocal(iq_w[:, co:co + cw],
                                             q_w[:, co:co + cw])
                        dotps = psmm.tile([128, cw], f32, tag="dotps")
                        prev = None
                        for ci, s in enumerate(csegs):
                            mm = nc.tensor.matmul(
                                dotps[:, ci * BQ:(ci + 1) * BQ],
                                lhs_dot_sb[b][:, s * 128:(s + 1) * 128],
                                rhs_dot[:, rq], start=(ci == 0),
                                stop=(ci == len(csegs) - 1))
                            if prev is not None:
                                add_dep_helper(mm.ins, prev.ins, sync=False,
                                               reason="psum group MM order")
                            prev = mm
                        p_eng.tensor_tensor(P_w[:, co:co + cw], dotps[:],
                                            iq_w[:, co:co + cw], alu.mult)

                    u_n = wp.tile([128, Fb_max], f16, tag="un")
                    nc.vector.tensor_scalar(u_n[:, :Fb], q_w[:, :Fb], 1.0, 1.0,
                                            alu.min, alu.subtract)
                    v_n = wp.tile([128, Fb_max], f16, tag="vn")
                    nc.vector.tensor_scalar(v_n[:, :Fb], q_w[:, :Fb], 0.5, 0.5,
                                            alu.min, alu.subtract)
                    u2 = wp.tile([128, Fb_max], f16, tag="u2")
                    u2_eng.activation(u2[:, :Fb], u_n[:, :Fb], AF.Square)
                    v2q = wp.tile([128, Fb_max], f16, tag="v2q")
                    v2_eng.tensor_tensor(v2q[:, :Fb], v_n[:, :Fb],
                                         v_n[:, :Fb], alu.mult)
                    nc.vector.tensor_scalar(v2q[:, :Fb], v2q[:, :Fb], 4.0,
                                            None, alu.mult)
                    G_w = wp.tile([128, Fb_max], f16, tag="G")
                    g_eng.tensor_tensor(G_w[:, :Fb], u2[:, :Fb],
                                        v2q[:, :Fb], alu.subtract)
                    u3n = wp.tile([128, Fb_max], f16, tag="u3n")
                    u3_eng.tensor_tensor(u3n[:, :Fb], u2[:, :Fb],
                                         u_n[:, :Fb], alu.mult)
                    v3n = wp.tile([128, Fb_max], f16, tag="v3n")
                    v3_eng.tensor_tensor(v3n[:, :Fb], v2q[:, :Fb],
                                         v_n[:, :Fb], alu.mult)
                    WX = wp.tile([128, Fb_max], f16, tag="WX")
                    wx_eng.tensor_tensor(WX[:, :Fb], v3n[:, :Fb],
                                         u3n[:, :Fb], alu.subtract)
                    T_w = wp.tile([128, Fb_max], f16, tag="T")
                    t_eng.tensor_tensor(T_w[:, :Fb], G_w[:, :Fb],
                                        P_w[:, :Fb], alu.mult)

                    # PE reduce streams
                    s1acc = psacc.tile([1, BQ], f32, tag="s1acc")
                    for s in range(ns):
                        cs = slice(s * BQ, (s + 1) * BQ)
                        nc.tensor.matmul(s1acc[:], W_V3, WX[:, cs],
                                         start=(s == 0), stop=(s == ns - 1))
                    s2acc = psacc.tile([1, BQ], f32, tag="s2acc")
                    for s in range(ns):
                        cs = slice(s * BQ, (s + 1) * BQ)
                        nc.tensor.matmul(s2acc[:], W_CH, T_w[:, cs],
                                         start=(s == 0), stop=(s == ns - 1))
                    # finalize rows (|.| folds into the final reduce)
                    nc.vector.tensor_scalar(zb2[0:1, rq], s1acc[:], -1.0,
                                            None, alu.add)
                    nc.vector.tensor_scalar(zb3[0:1, rq], s2acc[:], 1.0,
                                            None, alu.mult)

            # ---- final partial sums (|.| applied in-reduce) ----
            nc.vector.tensor_reduce(out_sb[0:1, 1:2], zb2[:],
                                    mybir.AxisListType.X, alu.add,
                                    apply_absolute_value=True)
            nc.vector.tensor_reduce(out_sb[0:1, 2:3], zb3[:],
                                    mybir.AxisListType.X, alu.add,
                                    apply_absolute_value=True)
            nc.sync.dma_start(d_out, out_sb[:])
    nc.compile()
    return nc


# ---------------------------------------------------------------- kernel
def prepare(inputs, reps=1):
    """Build (nc, in_maps, N) for the given inputs."""
    pred = np.asarray(inputs["pred"], dtype=np.float32)
    y = np.asarray(inputs["y"], dtype=np.float32)
    mid_pos = np.asarray(inputs["mid_pos"], dtype=np.float32)
    mid_vel = np.asarray(inputs["mid_vel"], dtype=np.float32)
    y_mean = np.asarray(inputs["y_mean"], dtype=np.float32)
    y_std = np.asarray(inputs["y_std"], dtype=np.float32)
    h = float(inputs["h"])
    vol = float(inputs["vol"])
    dt = float(inputs["dt"])
    nb = int(inputs["num_boundary_particles"])
    N = pred.shape[0]
    rows_core = N // NCORES

    y_inv = (y * y_std + y_mean).astype(np.float32)
    pos = mid_pos.copy()
    pos[nb:] += y_inv[nb:]
    vel = mid_vel.copy()
    vel[nb:] += (y_inv[nb:] / dt).astype(np.float32)

    perm, cand_lists = _build_structure(pos, h)
    pos_s = pos[perm]; vel_s = vel[perm]
    y_s = y[perm]; pred_s = pred[perm]

    nblk_total = N // BQ
    nblk_core = nblk_total // NCORES
    # size-balanced slot assignment: slot k gets the k-th octile by size
    order = np.argsort([-len(c) for c in cand_lists], kind="stable")
    slots = [order[k * NCORES:(k + 1) * NCORES] for k in range(nblk_core)]
    nseg_list = []
    for k in range(nblk_core):
        mx = max(len(cand_lists[b]) for b in slots[k])
        nseg_list.append(int(np.ceil(mx / 128)))

    key = (tuple(nseg_list), h, vol, N, reps)
    if key not in _PROGRAM_CACHE:
        _PROGRAM_CACHE[key] = _build_program(nseg_list, h, vol, rows_core,
                                             reps=reps)
    nc = _PROGRAM_CACHE[key]

    sigma = 8.0 / (np.pi * h ** 3)
    tsv = 2.0 * sigma * vol
    ch = -6.0 * sigma * vol / h
    l1w = (rows_core * 3) // 128
    inv_h = 1.0 / h

    in_maps = []
    for c in range(NCORES):
        m = {}
        qsel = []
        rhs_d2 = np.empty((13, rows_core), np.float16)
        rhs_dot = np.empty((8, rows_core), np.float16)
        for k in range(nblk_core):
            b = int(slots[k][c])
            qidx = np.arange(b * BQ, (b + 1) * BQ)
            qsel.append(qidx)
            ci = cand_lists[b]
            # block-local, h-scaled coordinates (fp16-friendly ranges)
            cb = pos_s[ci].mean(axis=0)
            vb = vel_s[ci].mean(axis=0)
            cpos = (pos_s[ci] - cb) * inv_h
            cvel = vel_s[ci] - vb
            csq = np.sum(cpos * cpos, axis=1, dtype=np.float64)
            cdiag = np.sum(cpos * cvel, axis=1)
            L = nseg_list[k] * 128
            npad = L - len(ci)
            cpos = np.concatenate([cpos,
                                   np.full((npad, 3), PAD_X, np.float32)])
            cvel = np.concatenate([cvel, np.zeros((npad, 3), np.float32)])
            csq = np.concatenate([csq, np.full(npad, 3 * PAD_X * PAD_X,
                                               np.float64)])
            cdiag = np.concatenate([cdiag, np.zeros(npad, np.float32)])
            # hi/lo fp16 splits: d2 = sqh_j+sql_j+sqh_i+sql_i+QB
            #                        - 2(xh_j.xh_i + xh_j.xl_i + xl_j.xh_i)
            cxh = cpos.astype(np.float16)
            cxl = (cpos - cxh.astype(np.float64)).astype(np.float16)
            csqh = csq.astype(np.float16)
            csql = (csq - csqh.astype(np.float64)).astype(np.float16)
            lhs_d2 = np.empty((13, L), np.float16)
            lhs_d2[0:3] = -2.0 * cxh.T
            lhs_d2[3:6] = -2.0 * cxh.T
            lhs_d2[6:9] = -2.0 * cxl.T
            lhs_d2[9] = csqh
            lhs_d2[10] = csql
            lhs_d2[11] = 1.0
            lhs_d2[12] = 1.0
            lhs_dot = np.empty((8, L), np.float16)
            lhs_dot[0:3] = cxh.T
            lhs_dot[3:6] = cvel.astype(np.float16).T
            lhs_dot[6] = cdiag.astype(np.float16)
            lhs_dot[7] = 1.0
            m[f"lhs_d2_{k}"] = lhs_d2
            m[f"lhs_dot_{k}"] = lhs_dot
            # query-side rows in the same local frame
            qpos = (pos_s[qidx] - cb) * inv_h
            qvel = vel_s[qidx] - vb
            qsq = np.sum(qpos * qpos, axis=1, dtype=np.float64)
            qdiag = np.sum(qpos * qvel, axis=1)
            qxh = qpos.astype(np.float16)
            qxl = (qpos - qxh.astype(np.float64)).astype(np.float16)
            qsqh = qsq.astype(np.float16)
            qsql = (qsq - qsqh.astype(np.float64) + QB).astype(np.float16)
            ks = slice(k * BQ, (k + 1) * BQ)
            rhs_d2[0:3, ks] = qxh.T
            rhs_d2[3:6, ks] = qxl.T
            rhs_d2[6:9, ks] = qxh.T
            rhs_d2[9, ks] = 1.0
            rhs_d2[10, ks] = 1.0
            rhs_d2[11, ks] = qsqh
            rhs_d2[12, ks] = qsql
            rhs_dot[0:3, ks] = qvel.astype(np.float16).T
            rhs_dot[3:6, ks] = qxh.T
            rhs_dot[6, ks] = -1.0
            rhs_dot[7, ks] = -qdiag.astype(np.float16)
        m["rhs_d2"] = rhs_d2
        m["rhs_dot"] = rhs_dot
        wcol = np.empty((128, 3), np.float32)
        wcol[:, 0] = -tsv
        wcol[:, 1] = tsv
        wcol[:, 2] = ch
        m["wcol"] = wcol.astype(np.float16)
        qidx_all = np.concatenate(qsel)
        m["y_t"] = y_s[qidx_all].reshape(128, l1w)
        m["pred_t"] = pred_s[qidx_all].reshape(128, l1w)
        in_maps.append(m)
    return nc, in_maps, N


def combine(results, N):
    parts = np.stack([results[c]["out"][0] for c in range(NCORES)])
    l1 = float(np.sum(parts[:, 0], dtype=np.float64))
    l2 = float(np.sum(parts[:, 1], dtype=np.float64))
    l3 = float(np.sum(parts[:, 2], dtype=np.float64))
    total = np.float32(1.0 * l1 / N) + np.float32(0.1) * np.float32(l2 / N) \
        + np.float32(0.1) * np.float32(l3 / N)
    return np.array(total, dtype=np.float32)


def kernel(**inputs):
    from concourse.bass_utils import run_bass_kernel_spmd
    nc, in_maps, N = prepare(inputs)
    res = run_bass_kernel_spmd(nc, in_maps, core_ids=list(range(NCORES)))
    global _last_results
    _last_results = res
    return combine(res.results, N)
